# revision 2
# baseline (speedup 1.0000x reference)
"""Attention-distillation KL loss on 8 Trainium2 NeuronCores.

Math: the reference softmaxes + L2-normalizes every row of student_out
[500000, 128], but the scalar loss only reads the rows gathered by
node_ids [256] and neighbor_idx [256, 32].  softmax and l2-normalize are
per-row, so they commute with the gather; furthermore
    sf = softmax(x) / ||softmax(x)|| = exp(x) / ||exp(x)||
(the softmax denominator and any max-shift cancel in the L2 norm), and
exp never overflows for N(0,1) logits.  So each core only has to:

  - exp the raw gathered rows,
  - compute cosine sims  sim[m,k] = <e_node[m], e_nbr[m,k]> / (||e_node[m]|| ||e_nbr[m,k]||),
  - masked log-softmax over k for student sims and teacher weights,
  - per-node KL.

Sharding: 256 sampled nodes -> 32 per core.  Per core the 32*32 = 1024
(m, k) pairs are laid out pair-major on SBUF partitions: 8 column bands
of [128 partitions x 128 classes]; pair q = m*32+k lives in band q//128,
partition q%128.  The node row is replicated across its 32 k-partitions
(host-side np.repeat), which makes every step a plain elementwise /
free-dim-reduce op - no transposes, no partition broadcasts.

Per-node reductions over k (32 partitions in a group) use one PE matmul
with a [128, 4] group-indicator matrix:  Z = G^T @ [ems | emt | w].
With  logZs/logZt  the masked-softmax denominators, per-node KL is
    kl[m] = (sum_k emt*(t - sim))/Zt + log(Zs/Zt)
(uses sum_k t_dist = 1).  Each core returns its 32 per-node KLs as a
[4, 8] tile; the host sums 256 values and divides by M.
"""

import numpy as np
from contextlib import ExitStack

import concourse.bass as bass
import concourse.tile as tile
from concourse import bacc, mybir
from concourse.bass_utils import run_bass_kernel_spmd

N_CORES = 8
M, K, C = 256, 32, 128
MPC = M // N_CORES            # nodes per core
PAIRS = MPC * K               # 1024 (m,k) pairs per core
T = PAIRS // 128              # 8 column bands
FREE = T * C                  # 1024 free-dim elements per partition
NGRP = 128 // K               # 4 nodes per band

_cache = {}


def _build_nc():
    nc = bacc.Bacc("TRN2", target_bir_lowering=False, debug=False,
                   enable_asserts=True, num_devices=N_CORES)
    f32 = mybir.dt.float32
    Exp = mybir.ActivationFunctionType.Exp
    Log = mybir.ActivationFunctionType.Ln

    xa = nc.dram_tensor("xa", [128, FREE], f32, kind="ExternalInput").ap()
    xb = nc.dram_tensor("xb", [128, FREE], f32, kind="ExternalInput").ap()
    tw = nc.dram_tensor("tw", [128, T], f32, kind="ExternalInput").ap()
    mk = nc.dram_tensor("mk", [128, T], f32, kind="ExternalInput").ap()
    gg = nc.dram_tensor("gg", [128, NGRP], f32, kind="ExternalInput").ap()
    kl = nc.dram_tensor("kl", [NGRP, T], f32, kind="ExternalOutput").ap()

    with tile.TileContext(nc) as tc, ExitStack() as ctx:
        sb = ctx.enter_context(tc.tile_pool(name="sb", bufs=1))
        ps = ctx.enter_context(tc.tile_pool(name="ps", bufs=1, space="PSUM"))

        sxa = sb.tile([128, FREE], f32)
        nc.sync.dma_start(sxa[:], xa[:, :])
        sxb = sb.tile([128, FREE], f32)
        nc.sync.dma_start(sxb[:], xb[:, :])
        stw = sb.tile([128, T], f32)
        nc.sync.dma_start(stw[:], tw[:, :])
        smk = sb.tile([128, T], f32)
        nc.sync.dma_start(smk[:], mk[:, :])
        sg = sb.tile([128, NGRP], f32)
        nc.sync.dma_start(sg[:], gg[:, :])

        ea = sb.tile([128, FREE], f32)
        nc.scalar.activation(ea[:], sxa[:], Exp)
        eb = sb.tile([128, FREE], f32)
        nc.scalar.activation(eb[:], sxb[:], Exp)

        # n2a = sum_c ea^2, n2b = sum_c eb^2, raw = sum_c ea*eb  (per band)
        prod = sb.tile([128, 3, FREE], f32)
        nc.vector.tensor_mul(prod[:, 0, :], ea[:], ea[:])
        nc.vector.tensor_mul(prod[:, 1, :], eb[:], eb[:])
        nc.vector.tensor_mul(prod[:, 2, :], ea[:], eb[:])
        red = sb.tile([128, 3 * T], f32)
        nc.vector.reduce_sum(
            red[:],
            prod[:, :, :].rearrange("p s (t c) -> p (s t) c", c=C),
            axis=mybir.AxisListType.X,
        )
        n2a, n2b, raw = red[:, 0:T], red[:, T:2 * T], red[:, 2 * T:3 * T]

        # rq = 1/sqrt(n2a*n2b) via exp(-0.5*log(nn)) + one Newton step.
        nn = sb.tile([128, T], f32)
        nc.vector.tensor_mul(nn[:], n2a, n2b)
        lg = sb.tile([128, T], f32)
        nc.scalar.activation(lg[:], nn[:], Log)
        rq = sb.tile([128, T], f32)
        nc.scalar.activation(rq[:], lg[:], Exp, scale=-0.5)
        t1 = sb.tile([128, T], f32)
        nc.vector.tensor_mul(t1[:], rq[:], rq[:])
        nc.vector.tensor_mul(t1[:], t1[:], nn[:])
        nc.vector.tensor_scalar(t1[:], t1[:], -0.5, 1.5,
                                op0=mybir.AluOpType.mult,
                                op1=mybir.AluOpType.add)
        nc.vector.tensor_mul(rq[:], rq[:], t1[:])

        sim = sb.tile([128, T], f32)
        nc.vector.tensor_mul(sim[:], raw, rq[:])

        # cat = [mask*exp(sim) | mask*exp(tw) | emt*(tw - sim)]
        cat = sb.tile([128, 3 * T], f32)
        es = sb.tile([128, T], f32)
        nc.scalar.activation(es[:], sim[:], Exp)
        nc.vector.tensor_mul(cat[:, 0:T], es[:], smk[:])
        et = sb.tile([128, T], f32)
        nc.scalar.activation(et[:], stw[:], Exp)
        nc.vector.tensor_mul(cat[:, T:2 * T], et[:], smk[:])
        dd = sb.tile([128, T], f32)
        nc.vector.tensor_sub(dd[:], stw[:], sim[:])
        nc.vector.tensor_mul(cat[:, 2 * T:3 * T], cat[:, T:2 * T], dd[:])

        # group-of-32-partitions sums:  [Zs | Zt | U] = G^T @ cat
        z = ps.tile([NGRP, 3 * T], f32)
        nc.tensor.matmul(z[:], sg[:], cat[:])
        zs, zt, u = z[:, 0:T], z[:, T:2 * T], z[:, 2 * T:3 * T]

        # kl[m] = U/Zt + log(Zs/Zt)
        rzt = sb.tile([NGRP, T], f32)
        nc.vector.reciprocal(rzt[:], zt)
        q1 = sb.tile([NGRP, T], f32)
        nc.vector.tensor_mul(q1[:], zs, rzt[:])
        lq = sb.tile([NGRP, T], f32)
        nc.scalar.activation(lq[:], q1[:], Log)
        out_t = sb.tile([NGRP, T], f32)
        nc.vector.tensor_mul(out_t[:], u, rzt[:])
        nc.vector.tensor_add(out_t[:], out_t[:], lq[:])
        nc.sync.dma_start(kl[:, :], out_t[:])

    nc.compile()
    return nc


def _get_nc():
    if "nc" not in _cache:
        _cache["nc"] = _build_nc()
    return _cache["nc"]


def _band_layout(a):
    """[PAIRS, C] row-major -> [128, T*C] band layout (band t cols hold
    pair rows 128t..128t+127)."""
    return np.ascontiguousarray(
        a.reshape(T, 128, C).transpose(1, 0, 2).reshape(128, FREE))


def _cols_layout(a):
    """[PAIRS] -> [128, T] with column t = pairs 128t..128t+127."""
    return np.ascontiguousarray(a.reshape(T, 128).T)


def _make_in_maps(student_out, teacher_weights, node_ids, neighbor_idx,
                  neighbor_mask):
    student_out = np.asarray(student_out, dtype=np.float32)
    teacher_weights = np.asarray(teacher_weights, dtype=np.float32)
    node_ids = np.asarray(node_ids).astype(np.int64)
    neighbor_idx = np.asarray(neighbor_idx).astype(np.int64)
    mask_f = np.asarray(neighbor_mask).astype(np.float32)

    gg = np.zeros((128, NGRP), dtype=np.float32)
    gg[np.arange(128), np.arange(128) // K] = 1.0

    in_maps = []
    for c in range(N_CORES):
        ms = slice(MPC * c, MPC * (c + 1))
        a_rows = student_out[neighbor_idx[ms].reshape(-1)]        # [1024, C]
        b_rows = np.repeat(student_out[node_ids[ms]], K, axis=0)  # [1024, C]
        in_maps.append({
            "xa": _band_layout(a_rows),
            "xb": _band_layout(b_rows),
            "tw": _cols_layout(teacher_weights[ms].reshape(-1)),
            "mk": _cols_layout(mask_f[ms].reshape(-1)),
            "gg": gg,
        })
    return in_maps


def _run(in_maps, **kwargs):
    return run_bass_kernel_spmd(_get_nc(), in_maps,
                                core_ids=list(range(N_CORES)), **kwargs)


def _per_node_kl(results):
    """results -> per-node kl [M] in node order."""
    kl = np.empty(M, dtype=np.float32)
    for c in range(N_CORES):
        t = results[c]["kl"]                      # [NGRP, T]; node = 4t+g
        kl[MPC * c: MPC * (c + 1)] = t.T.reshape(-1)
    return kl


def kernel(student_out, teacher_weights, node_ids, neighbor_idx,
           neighbor_mask):
    in_maps = _make_in_maps(student_out, teacher_weights, node_ids,
                            neighbor_idx, neighbor_mask)
    res = _run(in_maps)
    kl = _per_node_kl(res.results)
    return np.asarray(np.float64(kl.astype(np.float64).sum()) / M,
                      dtype=np.float32)


# revision 6
# speedup vs baseline: 1.1406x; 1.1406x over previous
"""Attention-distillation KL loss on 8 Trainium2 NeuronCores.

Math: the reference softmaxes + L2-normalizes every row of student_out
[500000, 128], but the scalar loss only reads the rows gathered by
node_ids [256] and neighbor_idx [256, 32].  softmax and l2-normalize are
per-row, so they commute with the gather; furthermore
    sf = softmax(x) / ||softmax(x)|| = exp(x) / ||exp(x)||
(the softmax denominator and any max-shift cancel in the L2 norm), and
exp never overflows for N(0,1) logits.  So each core only has to:

  - exp the raw gathered rows,
  - compute cosine sims  sim[m,k] = <e_node[m], e_nbr[m,k]> / (||e_node[m]|| ||e_nbr[m,k]||),
  - masked log-softmax over k for student sims and teacher weights,
  - per-node KL.

Sharding: 256 sampled nodes -> 32 per core.  Per core the 32*32 = 1024
(m, k) pairs are laid out pair-major on SBUF partitions: 8 column bands
of [128 partitions x 128 classes]; pair q = m*32+k lives in band q//128,
partition q%128.  The node row is replicated across its 32 k-partitions
(host-side np.repeat), which makes every step a plain elementwise /
free-dim-reduce op - no transposes, no partition broadcasts.

Per-node reductions over k (32 partitions in a group) use one PE matmul
with a [128, 4] group-indicator matrix:  Z = G^T @ [ems | emt | w].
With  logZs/logZt  the masked-softmax denominators, per-node KL is
    kl[m] = (sum_k emt*(t - sim))/Zt + log(Zs/Zt)
(uses sum_k t_dist = 1).  Each core returns its 32 per-node KLs as a
[4, 8] tile; the host sums 256 values and divides by M.
"""

import numpy as np
from contextlib import ExitStack

import concourse.bass as bass
import concourse.tile as tile
from concourse import bacc, mybir
from concourse.bass_utils import run_bass_kernel_spmd

N_CORES = 8
M, K, C = 256, 32, 128
MPC = M // N_CORES            # nodes per core
PAIRS = MPC * K               # 1024 (m,k) pairs per core
T = PAIRS // 128              # 8 column bands
FREE = T * C                  # 1024 free-dim elements per partition
NGRP = 128 // K               # 4 nodes per band

_cache = {}


def _patch_act_tables():
    """Make Exp/Ln/Square resolve only to the combined
    natural_log_exp_and_others table set, so the whole kernel needs a
    single ACT_TABLE_LOAD instead of thrashing exp<->ln sets (~1.3us per
    switch)."""
    if _cache.get("act_patched"):
        return
    orig = bacc.get_activation_tables
    combined = "natural_log_exp_and_others"
    special = {mybir.ActivationFunctionType.Exp,
               mybir.ActivationFunctionType.Ln,
               mybir.ActivationFunctionType.Square}

    def patched(arch):
        tabs = orig(arch)
        if combined in tabs and special <= tabs[combined]:
            for name, fns in tabs.items():
                if name != combined:
                    fns -= special
        return tabs

    bacc.get_activation_tables = patched
    _cache["act_patched"] = True


def _build_nc():
    _patch_act_tables()
    nc = bacc.Bacc("TRN2", target_bir_lowering=False, debug=False,
                   enable_asserts=True, num_devices=N_CORES)
    f32 = mybir.dt.float32
    Exp = mybir.ActivationFunctionType.Exp
    Log = mybir.ActivationFunctionType.Ln

    xa = nc.dram_tensor("xa", [128, FREE], f32, kind="ExternalInput").ap()
    xb = nc.dram_tensor("xb", [128, FREE], f32, kind="ExternalInput").ap()
    # sm packs [tw | mk | gg] -> one small DMA
    sm = nc.dram_tensor("sm", [128, 2 * T + NGRP], f32,
                        kind="ExternalInput").ap()
    kl = nc.dram_tensor("kl", [NGRP, T], f32, kind="ExternalOutput").ap()

    with tile.TileContext(nc) as tc, ExitStack() as ctx:
        sb = ctx.enter_context(tc.tile_pool(name="sb", bufs=1))
        ps = ctx.enter_context(tc.tile_pool(name="ps", bufs=1, space="PSUM"))

        sxa = sb.tile([128, FREE], f32)
        nc.sync.dma_start(sxa[:], xa[:, :])
        sxb = sb.tile([128, FREE], f32)
        nc.sync.dma_start(sxb[:], xb[:, :])
        ssm = sb.tile([128, 2 * T + NGRP], f32)
        nc.gpsimd.dma_start(ssm[:], sm[:, :])
        stw, smk, sg = ssm[:, 0:T], ssm[:, T:2 * T], ssm[:, 2 * T:]

        ea = sb.tile([128, FREE], f32)
        nc.scalar.activation(ea[:], sxa[:], Exp)
        eb = sb.tile([128, FREE], f32)
        nc.scalar.activation(eb[:], sxb[:], Exp)

        # n2a = sum_c ea^2, n2b = sum_c eb^2, raw = sum_c ea*eb  (per band)
        # squares on ScalarE (Square is in the same table set), cross
        # product on VectorE; three separate reduces so each starts as
        # soon as its product is ready.
        prod = sb.tile([128, 3, FREE], f32)
        nc.scalar.activation(prod[:, 0, :], ea[:],
                             mybir.ActivationFunctionType.Square)
        nc.scalar.activation(prod[:, 1, :], eb[:],
                             mybir.ActivationFunctionType.Square)
        nc.vector.tensor_mul(prod[:, 2, :], ea[:], eb[:])
        red = sb.tile([128, 3 * T], f32)
        for s in range(3):
            nc.vector.reduce_sum(
                red[:, s * T:(s + 1) * T],
                prod[:, s, :].rearrange("p (t c) -> p t c", c=C),
                axis=mybir.AxisListType.X,
            )
        n2a, n2b, raw = red[:, 0:T], red[:, T:2 * T], red[:, 2 * T:3 * T]

        # rq = 1/sqrt(n2a*n2b) via exp(-0.5*log(nn)) + one Newton step.
        nn = sb.tile([128, T], f32)
        nc.vector.tensor_mul(nn[:], n2a, n2b)
        lg = sb.tile([128, T], f32)
        nc.scalar.activation(lg[:], nn[:], Log)
        rq = sb.tile([128, T], f32)
        nc.scalar.activation(rq[:], lg[:], Exp, scale=-0.5)
        t1 = sb.tile([128, T], f32)
        nc.vector.tensor_mul(t1[:], rq[:], rq[:])
        nc.vector.tensor_mul(t1[:], t1[:], nn[:])
        nc.vector.tensor_scalar(t1[:], t1[:], -0.5, 1.5,
                                op0=mybir.AluOpType.mult,
                                op1=mybir.AluOpType.add)
        nc.vector.tensor_mul(rq[:], rq[:], t1[:])

        sim = sb.tile([128, T], f32)
        nc.vector.tensor_mul(sim[:], raw, rq[:])

        # cat = [mask*exp(sim) | mask*exp(tw) | emt*(tw - sim)]
        cat = sb.tile([128, 3 * T], f32)
        es = sb.tile([128, T], f32)
        nc.scalar.activation(es[:], sim[:], Exp)
        nc.vector.tensor_mul(cat[:, 0:T], es[:], smk[:])
        et = sb.tile([128, T], f32)
        nc.scalar.activation(et[:], stw[:], Exp)
        nc.vector.tensor_mul(cat[:, T:2 * T], et[:], smk[:])
        dd = sb.tile([128, T], f32)
        nc.vector.tensor_sub(dd[:], stw[:], sim[:])
        nc.vector.tensor_mul(cat[:, 2 * T:3 * T], cat[:, T:2 * T], dd[:])

        # group-of-32-partitions sums:  [Zs | Zt | U] = G^T @ cat
        z = ps.tile([NGRP, 3 * T], f32)
        nc.tensor.matmul(z[:], sg[:], cat[:])
        zs, zt, u = z[:, 0:T], z[:, T:2 * T], z[:, 2 * T:3 * T]

        # kl[m] = U/Zt + log(Zs/Zt)
        rzt = sb.tile([NGRP, T], f32)
        nc.vector.reciprocal(rzt[:], zt)
        q1 = sb.tile([NGRP, T], f32)
        nc.vector.tensor_mul(q1[:], zs, rzt[:])
        lq = sb.tile([NGRP, T], f32)
        nc.scalar.activation(lq[:], q1[:], Log)
        out_t = sb.tile([NGRP, T], f32)
        nc.vector.tensor_mul(out_t[:], u, rzt[:])
        nc.vector.tensor_add(out_t[:], out_t[:], lq[:])
        nc.sync.dma_start(kl[:, :], out_t[:])

    nc.compile()
    return nc


def _get_nc():
    if "nc" not in _cache:
        _cache["nc"] = _build_nc()
    return _cache["nc"]


def _band_layout(a):
    """[PAIRS, C] row-major -> [128, T*C] band layout (band t cols hold
    pair rows 128t..128t+127)."""
    return np.ascontiguousarray(
        a.reshape(T, 128, C).transpose(1, 0, 2).reshape(128, FREE))


def _cols_layout(a):
    """[PAIRS] -> [128, T] with column t = pairs 128t..128t+127."""
    return np.ascontiguousarray(a.reshape(T, 128).T)


def _make_in_maps(student_out, teacher_weights, node_ids, neighbor_idx,
                  neighbor_mask):
    student_out = np.asarray(student_out, dtype=np.float32)
    teacher_weights = np.asarray(teacher_weights, dtype=np.float32)
    node_ids = np.asarray(node_ids).astype(np.int64)
    neighbor_idx = np.asarray(neighbor_idx).astype(np.int64)
    mask_f = np.asarray(neighbor_mask).astype(np.float32)

    gg = np.zeros((128, NGRP), dtype=np.float32)
    gg[np.arange(128), np.arange(128) // K] = 1.0

    in_maps = []
    for c in range(N_CORES):
        ms = slice(MPC * c, MPC * (c + 1))
        a_rows = student_out[neighbor_idx[ms].reshape(-1)]        # [1024, C]
        b_rows = np.repeat(student_out[node_ids[ms]], K, axis=0)  # [1024, C]
        sm = np.concatenate([
            _cols_layout(teacher_weights[ms].reshape(-1)),
            _cols_layout(mask_f[ms].reshape(-1)),
            gg,
        ], axis=1)
        in_maps.append({
            "xa": _band_layout(a_rows),
            "xb": _band_layout(b_rows),
            "sm": np.ascontiguousarray(sm),
        })
    return in_maps


def _run(in_maps, **kwargs):
    return run_bass_kernel_spmd(_get_nc(), in_maps,
                                core_ids=list(range(N_CORES)), **kwargs)


def _per_node_kl(results):
    """results -> per-node kl [M] in node order."""
    kl = np.empty(M, dtype=np.float32)
    for c in range(N_CORES):
        t = results[c]["kl"]                      # [NGRP, T]; node = 4t+g
        kl[MPC * c: MPC * (c + 1)] = t.T.reshape(-1)
    return kl


def kernel(student_out, teacher_weights, node_ids, neighbor_idx,
           neighbor_mask):
    in_maps = _make_in_maps(student_out, teacher_weights, node_ids,
                            neighbor_idx, neighbor_mask)
    res = _run(in_maps)
    kl = _per_node_kl(res.results)
    return np.asarray(np.float64(kl.astype(np.float64).sum()) / M,
                      dtype=np.float32)
